# revision 3
# baseline (speedup 1.0000x reference)
"""Trainium2 Bass kernel for width-axis cross attention (sparse_attention problem).

reference semantics:
  Q = conv3x3(low1, w, b); K = conv3x3(low2, w, b)
  score[b,h,w,v] = sum_c Q[b,c,h,w] * K[b,c,h,v]
  A_left  = softmax(score, axis=-1)            (relu is identity on softmax)
  A_right = softmax(score^T, axis=-1)
  left  = low1 + einsum('bhwv,bchv->bchw', A_left,  low2)
  right = low2 + einsum('bhwv,bchv->bchw', A_right, low1)

Sharding: data-parallel over (batch, H-half) -> 8 shards, no cross-core comm.
"""

import os
import sys

for _p in ("/opt/trn_rl_repo", "/root/.axon_site/_ro/trn_rl_repo"):
    if os.path.isdir(_p) and _p not in sys.path:
        sys.path.append(_p)

import numpy as np
import ml_dtypes

import concourse.bacc as bacc
import concourse.bass as bass
import concourse.tile as tile
from concourse import mybir
from concourse import bass_utils

B, C, H, W = 4, 96, 192, 192
NCORES = 8
HL = H // 2          # local rows per core
WP = W + 2           # width-padded
HP = HL + 2          # local rows + halo
WC = W // 2          # 96-wide chunk (partition-dim chunks of the W axis)

F32 = mybir.dt.float32
F32R = mybir.dt.float32r
BF16 = mybir.dt.bfloat16
AF = mybir.ActivationFunctionType
ALU = mybir.AluOpType

_CACHE = {}


def _install_profile_hook():
    """Register the axon NTFF profiling hook (missing from this image's antenv)."""
    if _CACHE.get("hook_done"):
        return
    _CACHE["hook_done"] = True
    import types
    import antenv

    if "antenv.axon_hooks" not in sys.modules:
        mod = types.ModuleType("antenv.axon_hooks")
        _h = {"fn": None}
        mod.set_axon_ntff_profile_hook = lambda fn: _h.__setitem__("fn", fn)
        mod.get_axon_ntff_profile_hook = lambda: _h["fn"]
        sys.modules["antenv.axon_hooks"] = mod
        antenv.axon_hooks = mod
    mod = sys.modules["antenv.axon_hooks"]
    try:
        from trn_agent_boot.trn_boot import _ntff_profile_via_ctypes

        hook = _ntff_profile_via_ctypes("/opt/axon/libaxon_pjrt.so")
        if hook is not None:
            mod.set_axon_ntff_profile_hook(hook)
    except Exception as e:  # profiling is best-effort
        print(f"profile hook install failed: {e}", file=sys.stderr)
    # avoid remote artifact uploads from the profiling path
    bass_utils.upload_artifacts = lambda tmpdir: "local://" + str(tmpdir)


def _build():
    """Build + compile the per-core Bass module (identical on all 8 cores)."""
    nc = bacc.Bacc("TRN2", target_bir_lowering=False, debug=False,
                   num_devices=NCORES)

    # inputs (per core)
    x12 = nc.dram_tensor("x12", [C, HP, 2, WP], F32R, kind="ExternalInput").ap()
    xtb = nc.dram_tensor("xtb", [HL, WC, 4, C], BF16, kind="ExternalInput").ap()
    xt32 = nc.dram_tensor("xt32", [HL, WC, 4, C], F32, kind="ExternalInput").ap()
    wt = nc.dram_tensor("wt", [C, 9, C], F32R, kind="ExternalInput").ap()
    bias = nc.dram_tensor("bias", [C, 1], F32, kind="ExternalInput").ap()
    # outputs (transposed layout: host maps [h, w', wc, c] -> [c, h, wc*96+w'])
    loT = nc.dram_tensor("loT", [HL, WC, 2, C], F32, kind="ExternalOutput").ap()
    roT = nc.dram_tensor("roT", [HL, WC, 2, C], F32, kind="ExternalOutput").ap()

    with tile.TileContext(nc) as tc:
        with (
            tc.tile_pool(name="wpool", bufs=1) as wpool,
            tc.tile_pool(name="rows", bufs=6) as rows_pool,
            tc.tile_pool(name="xtbp", bufs=4) as xtb_pool,
            tc.tile_pool(name="xt32p", bufs=4) as xt32_pool,
            tc.tile_pool(name="qkp", bufs=3) as qk_pool,
            tc.tile_pool(name="ep", bufs=3) as e_pool,
            tc.tile_pool(name="rsp", bufs=4) as rs_pool,
            tc.tile_pool(name="outp", bufs=4) as out_pool,
            tc.tile_pool(name="convps", bufs=2, space="PSUM") as conv_pp,
            tc.tile_pool(name="sps", bufs=2, space="PSUM") as s_pp,
            tc.tile_pool(name="stps", bufs=2, space="PSUM") as st_pp,
            tc.tile_pool(name="mps", bufs=2, space="PSUM") as m_pp,
        ):
            wt_s = wpool.tile([C, 9, C], F32R)
            nc.sync.dma_start(wt_s[:], wt)
            bias_s = wpool.tile([C, 1], F32)
            nc.sync.dma_start(bias_s[:], bias)

            row_t = {}

            def load_row(r):
                t = rows_pool.tile([C, 2, WP], F32R, tag="row")
                nc.sync.dma_start(t[:], x12[:, r, :, :])
                row_t[r] = t

            for r in range(3):
                load_row(r)

            for h in range(HL):
                if h + 3 <= HP - 1:
                    load_row(h + 3)

                xtb_t = xtb_pool.tile([WC, 4, C], BF16)
                nc.sync.dma_start(xtb_t[:], xtb[h])
                xt32_t = xt32_pool.tile([WC, 4, C], F32)
                nc.sync.dma_start(xt32_t[:], xt32[h])

                # --- conv3x3 for Q|K (both tensors in one rhs stream) ---
                conv_ps = conv_pp.tile([C, 2, W], F32)
                for t in range(9):
                    ky, kx = t // 3, t % 3
                    nc.tensor.matmul(
                        conv_ps[:],
                        wt_s[:, t, :],
                        row_t[h + ky][:, :, kx:kx + W],
                        start=(t == 0), stop=(t == 8),
                    )
                qk = qk_pool.tile([C, 2, W], BF16)
                nc.scalar.activation(qk[:], conv_ps[:], AF.Identity,
                                     bias=bias_s[:], scale=1.0)

                # --- S = Q^T K  and St = K^T Q, in 96-row chunks ---
                s_ps = s_pp.tile([WC, 2, W], F32)
                st_ps = st_pp.tile([WC, 2, W], F32)
                for wc in range(2):
                    nc.tensor.matmul(s_ps[:, wc, :],
                                     qk[:, 0, bass.ts(wc, WC)], qk[:, 1, :],
                                     start=True, stop=True)
                    nc.tensor.matmul(st_ps[:, wc, :],
                                     qk[:, 1, bass.ts(wc, WC)], qk[:, 0, :],
                                     start=True, stop=True)

                # --- E = exp(S), Et = exp(St), with fused row-sums ---
                e_t = e_pool.tile([WC, 4, W], BF16)
                rs_t = rs_pool.tile([WC, 4], F32)
                for wc in range(2):
                    nc.scalar.activation(e_t[:, wc, :], s_ps[:, wc, :], AF.Exp,
                                         accum_out=rs_t[:, wc:wc + 1])
                    nc.scalar.activation(e_t[:, 2 + wc, :], st_ps[:, wc, :], AF.Exp,
                                         accum_out=rs_t[:, 2 + wc:3 + wc])
                rcp_t = rs_pool.tile([WC, 4], F32, tag="rcp")
                nc.vector.reciprocal(rcp_t[:], rs_t[:])

                # --- apply attention (unnormalized), output transposed [w, c] ---
                # left chunk wc:  M_l[w,c] = sum_v Et[v, w] * low2T[v, c]
                # right chunk vc: M_r[v,c] = sum_w E[w, v] * low1T[w, c]
                m_ps = m_pp.tile([WC, 4, C], F32)
                for wc in range(2):
                    for vc in range(2):
                        nc.tensor.matmul(m_ps[:, wc, :],
                                         e_t[:, 2 + vc, bass.ts(wc, WC)],
                                         xtb_t[:, 2 + vc, :],
                                         start=(vc == 0), stop=(vc == 1))
                for vc in range(2):
                    for wc in range(2):
                        nc.tensor.matmul(m_ps[:, 2 + vc, :],
                                         e_t[:, wc, bass.ts(vc, WC)],
                                         xtb_t[:, wc, :],
                                         start=(wc == 0), stop=(wc == 1))

                # --- finalize: out = base^T + M * (1/rs)  (per-partition scalar) ---
                lo_t = out_pool.tile([WC, 2, C], F32, tag="lo")
                ro_t = out_pool.tile([WC, 2, C], F32, tag="ro")
                for wc in range(2):
                    nc.vector.scalar_tensor_tensor(
                        lo_t[:, wc, :], m_ps[:, wc, :], rcp_t[:, wc:wc + 1],
                        xt32_t[:, wc, :], op0=ALU.mult, op1=ALU.add)
                    nc.vector.scalar_tensor_tensor(
                        ro_t[:, wc, :], m_ps[:, 2 + wc, :], rcp_t[:, 2 + wc:3 + wc],
                        xt32_t[:, 2 + wc, :], op0=ALU.mult, op1=ALU.add)

                nc.sync.dma_start(loT[h], lo_t[:])
                nc.sync.dma_start(roT[h], ro_t[:])

    nc.compile()
    return nc


def _prepare_inputs(low1, low2, conv_w, conv_b):
    low1 = np.asarray(low1, dtype=np.float32)
    low2 = np.asarray(low2, dtype=np.float32)
    conv_w = np.asarray(conv_w, dtype=np.float32)
    conv_b = np.asarray(conv_b, dtype=np.float32)

    xp1 = np.zeros((B, C, H + 2, W + 2), np.float32)
    xp1[:, :, 1:-1, 1:-1] = low1
    xp2 = np.zeros((B, C, H + 2, W + 2), np.float32)
    xp2[:, :, 1:-1, 1:-1] = low2

    wt = np.ascontiguousarray(conv_w.transpose(1, 2, 3, 0).reshape(C, 9, C))
    bias = np.ascontiguousarray(conv_b.reshape(C, 1))

    in_maps = []
    for k in range(NCORES):
        b, half = k // 2, k % 2
        r0 = half * HL
        sl1 = xp1[b, :, r0:r0 + HP, :]
        sl2 = xp2[b, :, r0:r0 + HP, :]
        x12 = np.ascontiguousarray(np.stack([sl1, sl2], axis=2))

        # transposed views [h, w', wc, c] for both tensors
        l1t = low1[b, :, r0:r0 + HL, :].transpose(1, 2, 0)   # [h, w, c]
        l2t = low2[b, :, r0:r0 + HL, :].transpose(1, 2, 0)
        a1 = l1t.reshape(HL, 2, WC, C).transpose(0, 2, 1, 3)  # [h, w', wc, c]
        a2 = l2t.reshape(HL, 2, WC, C).transpose(0, 2, 1, 3)
        xt = np.concatenate([a1, a2], axis=2)                 # [h, w', 4, c]
        xt32 = np.ascontiguousarray(xt)
        xtb = xt32.astype(ml_dtypes.bfloat16)

        in_maps.append({
            "x12": x12,
            "xtb": xtb,
            "xt32": xt32,
            "wt": wt,
            "bias": bias,
        })
    return in_maps


def _assemble(results):
    left = np.empty((B, C, H, W), np.float32)
    right = np.empty((B, C, H, W), np.float32)
    for k in range(NCORES):
        b, half = k // 2, k % 2
        r0 = half * HL
        for name, dst in (("loT", left), ("roT", right)):
            arr = results[k][name]                    # [h, w', wc, c]
            dst[b, :, r0:r0 + HL, :] = (
                arr.transpose(3, 0, 2, 1).reshape(C, HL, W))
    return left, right


def _run(inputs, trace=False):
    if trace:
        _install_profile_hook()
    if "nc" not in _CACHE:
        _CACHE["nc"] = _build()
    nc = _CACHE["nc"]
    in_maps = _prepare_inputs(**inputs)
    res = bass_utils.run_bass_kernel_spmd(
        nc, in_maps, core_ids=list(range(NCORES)), trace=trace)
    left, right = _assemble(res.results)
    return (left, right), res


def kernel(**inputs):
    out, _ = _run(inputs, trace=False)
    return out


# revision 8
# speedup vs baseline: 1.2401x; 1.2401x over previous
"""Trainium2 Bass kernel for width-axis cross attention (sparse_attention problem).

reference semantics:
  Q = conv3x3(low1, w, b); K = conv3x3(low2, w, b)
  score[b,h,w,v] = sum_c Q[b,c,h,w] * K[b,c,h,v]
  A_left  = softmax(score, axis=-1)            (relu is identity on softmax)
  A_right = softmax(score^T, axis=-1)
  left  = low1 + einsum('bhwv,bchv->bchw', A_left,  low2)
  right = low2 + einsum('bhwv,bchv->bchw', A_right, low1)

Sharding: data-parallel over (batch, H-half) -> 8 shards, no cross-core comm.

Per-core dataflow (96 rows, processed in row pairs):
 - conv as 9 accumulating f32r matmuls per tensor, 2 output rows per matmul
   (shared weight loads), PSUM -> SBUF bf16 with bias via ScalarE.
 - S = Q^T K and St = K^T Q in bf16; exp via ScalarE (one [96,384] op per side).
 - apply matmuls in bf16 against host-pre-transposed inputs, with an extra
   ones-column producing the softmax row-sums for free.
 - finalize = (M * 1/rs) + base^T in one fused VectorE op; outputs stored
   width-transposed, host un-transposes.
"""

import os
import sys

for _p in ("/opt/trn_rl_repo", "/root/.axon_site/_ro/trn_rl_repo"):
    if os.path.isdir(_p) and _p not in sys.path:
        sys.path.append(_p)

import numpy as np
import ml_dtypes

import concourse.bacc as bacc
import concourse.bass as bass
import concourse.tile as tile
from concourse import mybir
from concourse import bass_utils

B, C, H, W = 4, 96, 192, 192
NCORES = 8
HL = H // 2          # local rows per core
WP = W + 2           # width-padded
WC = W // 2          # 96-wide chunk of the W axis
NPAIR = HL // 2      # 48 row pairs
PAIRS_PER_CHUNK = 7
NCHUNK = -(-NPAIR // PAIRS_PER_CHUNK)        # 7
CROWS = 2 * PAIRS_PER_CHUNK + 2              # 16 rows per input chunk (1 halo each side)

F32 = mybir.dt.float32
F32R = mybir.dt.float32r
BF16 = mybir.dt.bfloat16
AF = mybir.ActivationFunctionType
ALU = mybir.AluOpType

_CACHE = {}


def _install_profile_hook():
    """Register the axon NTFF profiling hook (missing from this image's antenv)."""
    if _CACHE.get("hook_done"):
        return
    _CACHE["hook_done"] = True
    import types
    import antenv

    if "antenv.axon_hooks" not in sys.modules:
        mod = types.ModuleType("antenv.axon_hooks")
        _h = {"fn": None}
        mod.set_axon_ntff_profile_hook = lambda fn: _h.__setitem__("fn", fn)
        mod.get_axon_ntff_profile_hook = lambda: _h["fn"]
        sys.modules["antenv.axon_hooks"] = mod
        antenv.axon_hooks = mod
    mod = sys.modules["antenv.axon_hooks"]
    try:
        from trn_agent_boot.trn_boot import _ntff_profile_via_ctypes

        hook = _ntff_profile_via_ctypes("/opt/axon/libaxon_pjrt.so")
        if hook is not None:
            mod.set_axon_ntff_profile_hook(hook)
    except Exception as e:  # profiling is best-effort
        print(f"profile hook install failed: {e}", file=sys.stderr)
    # avoid remote artifact uploads from the profiling path
    bass_utils.upload_artifacts = lambda tmpdir: "local://" + str(tmpdir)


def _build():
    """Build + compile the per-core Bass module (identical on all 8 cores)."""
    nc = bacc.Bacc("TRN2", target_bir_lowering=False, debug=False,
                   num_devices=NCORES)

    # inputs (per core)
    x1c = nc.dram_tensor("x1c", [NCHUNK, C, CROWS, WP], F32R,
                         kind="ExternalInput").ap()
    x2c = nc.dram_tensor("x2c", [NCHUNK, C, CROWS, WP], F32R,
                         kind="ExternalInput").ap()
    # [pair, w', row, slot, col]; slots 0,1 = low1T w-chunks, 2,3 = low2T
    # xtb has a 97th all-ones column (bf16) for free softmax row-sums.
    xtb = nc.dram_tensor("xtb", [NPAIR, WC, 2, 4, WC + 1], BF16,
                         kind="ExternalInput").ap()
    xt32 = nc.dram_tensor("xt32", [NPAIR, WC, 2, 4, WC], F32,
                          kind="ExternalInput").ap()
    wt = nc.dram_tensor("wt", [C, 9, C], F32R, kind="ExternalInput").ap()
    bias = nc.dram_tensor("bias", [C, 1], F32, kind="ExternalInput").ap()
    # outputs, transposed layout: [pair, w', row, wc, c]
    loT = nc.dram_tensor("loT", [NPAIR, WC, 2, 2, C], F32,
                         kind="ExternalOutput").ap()
    roT = nc.dram_tensor("roT", [NPAIR, WC, 2, 2, C], F32,
                         kind="ExternalOutput").ap()

    with tile.TileContext(nc) as tc:
        with (
            tc.tile_pool(name="wpool", bufs=1) as wpool,
            tc.tile_pool(name="chunks", bufs=2) as chunk_pool,
            tc.tile_pool(name="xtbp", bufs=2) as xtb_pool,
            tc.tile_pool(name="xt32p", bufs=2) as xt32_pool,
            tc.tile_pool(name="qkp", bufs=2) as qk_pool,
            tc.tile_pool(name="ep", bufs=3) as e_pool,
            tc.tile_pool(name="rcpp", bufs=3) as rcp_pool,
            tc.tile_pool(name="outp", bufs=3) as out_pool,
            tc.tile_pool(name="convps", bufs=1, space="PSUM") as conv_pp,
            tc.tile_pool(name="sps", bufs=2, space="PSUM") as s_pp,
            tc.tile_pool(name="stps", bufs=2, space="PSUM") as st_pp,
            tc.tile_pool(name="mps", bufs=2, space="PSUM") as m_pp,
        ):
            wt_s = wpool.tile([C, 9, C], F32R)
            nc.sync.dma_start(wt_s[:], wt)
            bias_s = wpool.tile([C, 1], F32)
            nc.sync.dma_start(bias_s[:], bias)

            ch_t = {}

            def load_chunk(j):
                t1 = chunk_pool.tile([C, CROWS, WP], F32R, tag="x1")
                nc.sync.dma_start(t1[:], x1c[j])
                t2 = chunk_pool.tile([C, CROWS, WP], F32R, tag="x2")
                nc.sync.dma_start(t2[:], x2c[j])
                ch_t[j] = (t1, t2)

            load_chunk(0)

            for j in range(NCHUNK):
                if j + 1 < NCHUNK:
                    load_chunk(j + 1)
                t1, t2 = ch_t[j]
                npairs = min(PAIRS_PER_CHUNK, NPAIR - j * PAIRS_PER_CHUNK)
                for p in range(npairs):
                    q = j * PAIRS_PER_CHUNK + p   # global pair index

                    xtb_t = xtb_pool.tile([WC, 2, 4, WC + 1], BF16)
                    nc.sync.dma_start(xtb_t[:], xtb[q])
                    xt32_t = xt32_pool.tile([WC, 2, 4, WC], F32)
                    nc.sync.dma_start(xt32_t[:], xt32[q])

                    # --- conv3x3 for both rows of the pair, Q and K ---
                    q_ps = conv_pp.tile([C, 2, W], F32, tag="q2")
                    k_ps = conv_pp.tile([C, 2, W], F32, tag="k2")
                    for t in range(9):
                        ky, kx = t // 3, t % 3
                        r = 2 * p + ky
                        nc.tensor.matmul(q_ps[:], wt_s[:, t, :],
                                         t1[:, r:r + 2, kx:kx + W],
                                         start=(t == 0), stop=(t == 8))
                        nc.tensor.matmul(k_ps[:], wt_s[:, t, :],
                                         t2[:, r:r + 2, kx:kx + W],
                                         start=(t == 0), stop=(t == 8))
                    qk = qk_pool.tile([C, 2, 2, W], BF16)   # [c, row, q/k, w]
                    nc.scalar.activation(qk[:, :, 0, :], q_ps[:], AF.Identity,
                                         bias=bias_s[:], scale=1.0)
                    nc.scalar.activation(qk[:, :, 1, :], k_ps[:], AF.Identity,
                                         bias=bias_s[:], scale=1.0)

                    lo_t = out_pool.tile([WC, 2, 2, C], F32, tag="lo")
                    ro_t = out_pool.tile([WC, 2, 2, C], F32, tag="ro")
                    for rr in range(2):
                        # --- S = Q^T K and St = K^T Q, chunked over W ---
                        s_ps = s_pp.tile([WC, 2, W], F32)
                        st_ps = st_pp.tile([WC, 2, W], F32)
                        for wc in range(2):
                            nc.tensor.matmul(s_ps[:, wc, :],
                                             qk[:, rr, 0, bass.ts(wc, WC)],
                                             qk[:, rr, 1, :],
                                             start=True, stop=True)
                            nc.tensor.matmul(st_ps[:, wc, :],
                                             qk[:, rr, 1, bass.ts(wc, WC)],
                                             qk[:, rr, 0, :],
                                             start=True, stop=True)

                        # --- E = exp(S), Et = exp(St) ---
                        e_t = e_pool.tile([WC, 4, W], BF16)
                        nc.scalar.activation(e_t[:, 0:2, :], s_ps[:], AF.Exp)
                        nc.scalar.activation(e_t[:, 2:4, :], st_ps[:], AF.Exp)

                        # --- apply (unnormalized) + ones-column row-sums ---
                        m_ps = m_pp.tile([WC, 4, WC + 1], F32)
                        for wc in range(2):
                            for vc in range(2):
                                nc.tensor.matmul(
                                    m_ps[:, wc, :],
                                    e_t[:, 2 + vc, bass.ts(wc, WC)],
                                    xtb_t[:, rr, 2 + vc, :],
                                    start=(vc == 0), stop=(vc == 1))
                        for vc in range(2):
                            for wc in range(2):
                                nc.tensor.matmul(
                                    m_ps[:, 2 + vc, :],
                                    e_t[:, wc, bass.ts(vc, WC)],
                                    xtb_t[:, rr, wc, :],
                                    start=(wc == 0), stop=(wc == 1))

                        rcp_t = rcp_pool.tile([WC, 4], F32)
                        nc.vector.reciprocal(rcp_t[:], m_ps[:, :, WC:WC + 1])

                        # --- finalize: out = base^T + M * (1/rs) ---
                        for wc in range(2):
                            nc.vector.scalar_tensor_tensor(
                                lo_t[:, rr, wc, :], m_ps[:, wc, 0:C],
                                rcp_t[:, wc:wc + 1], xt32_t[:, rr, wc, :],
                                op0=ALU.mult, op1=ALU.add)
                            nc.vector.scalar_tensor_tensor(
                                ro_t[:, rr, wc, :], m_ps[:, 2 + wc, 0:C],
                                rcp_t[:, 2 + wc:3 + wc], xt32_t[:, rr, 2 + wc, :],
                                op0=ALU.mult, op1=ALU.add)
                    out_eng = (nc.sync if os.environ.get("KV2_SYNCOUT")
                               else nc.gpsimd)
                    out_eng.dma_start(loT[q], lo_t[:])
                    out_eng.dma_start(roT[q], ro_t[:])

    nc.compile()
    return nc


def _prepare_inputs(low1, low2, conv_w, conv_b):
    low1 = np.asarray(low1, dtype=np.float32)
    low2 = np.asarray(low2, dtype=np.float32)
    conv_w = np.asarray(conv_w, dtype=np.float32)
    conv_b = np.asarray(conv_b, dtype=np.float32)

    xp1 = np.zeros((B, C, H + 2, W + 2), np.float32)
    xp1[:, :, 1:-1, 1:-1] = low1
    xp2 = np.zeros((B, C, H + 2, W + 2), np.float32)
    xp2[:, :, 1:-1, 1:-1] = low2

    wt = np.ascontiguousarray(conv_w.transpose(1, 2, 3, 0).reshape(C, 9, C))
    bias = np.ascontiguousarray(conv_b.reshape(C, 1))

    in_maps = []
    for k in range(NCORES):
        b, half = k // 2, k % 2
        r0 = half * HL

        def make_chunks(xp):
            out = np.zeros((NCHUNK, C, CROWS, WP), np.float32)
            for j in range(NCHUNK):
                lo = r0 + 14 * j
                hi = min(lo + CROWS, H + 2)
                out[j, :, :hi - lo, :] = xp[b, :, lo:hi, :]
            return out

        x1ck = make_chunks(xp1)
        x2ck = make_chunks(xp2)

        # transposed [h, w', slot, c] for both tensors; slot 0,1=low1T, 2,3=low2T
        l1t = low1[b, :, r0:r0 + HL, :].transpose(1, 2, 0)   # [h, w, c]
        l2t = low2[b, :, r0:r0 + HL, :].transpose(1, 2, 0)
        a1 = l1t.reshape(HL, 2, WC, C).transpose(0, 2, 1, 3)  # [h, w', wc, c]
        a2 = l2t.reshape(HL, 2, WC, C).transpose(0, 2, 1, 3)
        xt = np.concatenate([a1, a2], axis=2)                 # [h, w', 4, c]
        # pair-batch: [pair, w', row, slot, c]
        xt32 = np.ascontiguousarray(
            xt.reshape(NPAIR, 2, WC, 4, C).transpose(0, 2, 1, 3, 4))
        xtb = np.concatenate(
            [xt32, np.ones((NPAIR, WC, 2, 4, 1), np.float32)],
            axis=4).astype(ml_dtypes.bfloat16)

        in_maps.append({
            "x1c": x1ck,
            "x2c": x2ck,
            "xtb": np.ascontiguousarray(xtb),
            "xt32": xt32,
            "wt": wt,
            "bias": bias,
        })
    return in_maps


def _assemble(results):
    left = np.empty((B, C, H, W), np.float32)
    right = np.empty((B, C, H, W), np.float32)
    for k in range(NCORES):
        b, half = k // 2, k % 2
        r0 = half * HL
        for name, dst in (("loT", left), ("roT", right)):
            arr = results[k][name]                  # [pair, w', row, wc, c]
            # -> [c, pair, row, wc, w'] -> [c, h, w]
            dst[b, :, r0:r0 + HL, :] = (
                arr.transpose(4, 0, 2, 3, 1).reshape(C, HL, W))
    return left, right


def _run(inputs, trace=False):
    if trace:
        _install_profile_hook()
    if "nc" not in _CACHE:
        _CACHE["nc"] = _build()
    nc = _CACHE["nc"]
    in_maps = _prepare_inputs(**inputs)
    res = bass_utils.run_bass_kernel_spmd(
        nc, in_maps, core_ids=list(range(NCORES)), trace=trace)
    left, right = _assemble(res.results)
    return (left, right), res


def kernel(**inputs):
    out, _ = _run(inputs, trace=False)
    return out


# revision 11
# speedup vs baseline: 1.3067x; 1.0537x over previous
"""Trainium2 Bass kernel for width-axis cross attention (sparse_attention problem).

reference semantics:
  Q = conv3x3(low1, w, b); K = conv3x3(low2, w, b)
  score[b,h,w,v] = sum_c Q[b,c,h,w] * K[b,c,h,v]
  A_left  = softmax(score, axis=-1)            (relu is identity on softmax)
  A_right = softmax(score^T, axis=-1)
  left  = low1 + einsum('bhwv,bchv->bchw', A_left,  low2)
  right = low2 + einsum('bhwv,bchv->bchw', A_right, low1)

Sharding: data-parallel over (batch, H-half) -> 8 shards, no cross-core comm.

Per-core dataflow (96 rows, processed in row pairs):
 - conv as 9 accumulating f32r matmuls per tensor, 2 output rows per matmul
   (shared weight loads), PSUM -> SBUF bf16 with bias via ScalarE.
 - S = Q^T K and St = K^T Q in bf16; exp via ScalarE (one [96,384] op per side).
 - apply matmuls in bf16 against host-pre-transposed inputs, with an extra
   ones-column producing the softmax row-sums for free.
 - finalize = (M * 1/rs) + base^T in one fused VectorE op; outputs stored
   width-transposed, host un-transposes.
"""

import os
import sys

for _p in ("/opt/trn_rl_repo", "/root/.axon_site/_ro/trn_rl_repo"):
    if os.path.isdir(_p) and _p not in sys.path:
        sys.path.append(_p)

import numpy as np
import ml_dtypes

import concourse.bacc as bacc
import concourse.bass as bass
import concourse.tile as tile
from concourse import mybir
from concourse import bass_utils

B, C, H, W = 4, 96, 192, 192
NCORES = 8
HL = H // 2          # local rows per core
WP = W + 2           # width-padded
WC = W // 2          # 96-wide chunk of the W axis
NPAIR = HL // 2      # 48 row pairs
PAIRS_PER_CHUNK = 7
NCHUNK = -(-NPAIR // PAIRS_PER_CHUNK)        # 7
CROWS = 2 * PAIRS_PER_CHUNK + 2              # 16 rows per input chunk (1 halo each side)

F32 = mybir.dt.float32
F32R = mybir.dt.float32r
BF16 = mybir.dt.bfloat16
AF = mybir.ActivationFunctionType
ALU = mybir.AluOpType

_CACHE = {}


def _install_profile_hook():
    """Register the axon NTFF profiling hook (missing from this image's antenv)."""
    if _CACHE.get("hook_done"):
        return
    _CACHE["hook_done"] = True
    import types
    import antenv

    if "antenv.axon_hooks" not in sys.modules:
        mod = types.ModuleType("antenv.axon_hooks")
        _h = {"fn": None}
        mod.set_axon_ntff_profile_hook = lambda fn: _h.__setitem__("fn", fn)
        mod.get_axon_ntff_profile_hook = lambda: _h["fn"]
        sys.modules["antenv.axon_hooks"] = mod
        antenv.axon_hooks = mod
    mod = sys.modules["antenv.axon_hooks"]
    try:
        from trn_agent_boot.trn_boot import _ntff_profile_via_ctypes

        hook = _ntff_profile_via_ctypes("/opt/axon/libaxon_pjrt.so")
        if hook is not None:
            mod.set_axon_ntff_profile_hook(hook)
    except Exception as e:  # profiling is best-effort
        print(f"profile hook install failed: {e}", file=sys.stderr)
    # avoid remote artifact uploads from the profiling path
    bass_utils.upload_artifacts = lambda tmpdir: "local://" + str(tmpdir)


def _build():
    """Build + compile the per-core Bass module (identical on all 8 cores)."""
    nc = bacc.Bacc("TRN2", target_bir_lowering=False, debug=False,
                   num_devices=NCORES)

    # inputs (per core)
    x1c = nc.dram_tensor("x1c", [NCHUNK, C, CROWS, WP], F32R,
                         kind="ExternalInput").ap()
    x2c = nc.dram_tensor("x2c", [NCHUNK, C, CROWS, WP], F32R,
                         kind="ExternalInput").ap()
    # [pair, w', row, slot, col]; slots 0,1 = low1T w-chunks, 2,3 = low2T
    # xtb has a 97th all-ones column (bf16) for free softmax row-sums.
    xtb = nc.dram_tensor("xtb", [NPAIR, WC, 2, 4, WC + 1], BF16,
                         kind="ExternalInput").ap()
    xt32 = nc.dram_tensor("xt32", [NPAIR, WC, 2, 4, WC], F32,
                          kind="ExternalInput").ap()
    wt = nc.dram_tensor("wt", [C, 9, C], F32R, kind="ExternalInput").ap()
    bias = nc.dram_tensor("bias", [C, 1], F32, kind="ExternalInput").ap()
    # outputs, transposed layout: [pair, w', row, wc, c]
    loT = nc.dram_tensor("loT", [NPAIR, WC, 2, 2, C], F32,
                         kind="ExternalOutput").ap()
    roT = nc.dram_tensor("roT", [NPAIR, WC, 2, 2, C], F32,
                         kind="ExternalOutput").ap()

    with tile.TileContext(nc) as tc:
        with (
            tc.tile_pool(name="wpool", bufs=1) as wpool,
            tc.tile_pool(name="chunks", bufs=2) as chunk_pool,
            tc.tile_pool(name="xtbp", bufs=3) as xtb_pool,
            tc.tile_pool(name="xt32p", bufs=3) as xt32_pool,
            tc.tile_pool(name="qkp", bufs=2) as qk_pool,
            tc.tile_pool(name="ep", bufs=3) as e_pool,
            tc.tile_pool(name="rcpp", bufs=3) as rcp_pool,
            tc.tile_pool(name="outp", bufs=3) as out_pool,
            tc.tile_pool(name="convps", bufs=1, space="PSUM") as conv_pp,
            tc.tile_pool(name="sps", bufs=2, space="PSUM") as s_pp,
            tc.tile_pool(name="stps", bufs=2, space="PSUM") as st_pp,
            tc.tile_pool(name="mps", bufs=2, space="PSUM") as m_pp,
        ):
            wt_s = wpool.tile([C, 9, C], F32R)
            nc.sync.dma_start(wt_s[:], wt)
            bias_s = wpool.tile([C, 1], F32)
            nc.sync.dma_start(bias_s[:], bias)

            ch_t = {}

            def load_chunk(j):
                t1 = chunk_pool.tile([C, CROWS, WP], F32R, tag="x1")
                nc.sync.dma_start(t1[:], x1c[j])
                t2 = chunk_pool.tile([C, CROWS, WP], F32R, tag="x2")
                nc.sync.dma_start(t2[:], x2c[j])
                ch_t[j] = (t1, t2)

            load_chunk(0)
            state = {}

            def emit_conv(q):
                """conv3x3 + bias/cast for pair q; prefetch DMAs for pair q."""
                j, p = divmod(q, PAIRS_PER_CHUNK)
                if p == 0 and j + 1 < NCHUNK:
                    load_chunk(j + 1)
                t1, t2 = ch_t[j]

                xtb_t = xtb_pool.tile([WC, 2, 4, WC + 1], BF16)
                nc.sync.dma_start(xtb_t[:], xtb[q])
                xt32_t = xt32_pool.tile([WC, 2, 4, WC], F32)
                nc.sync.dma_start(xt32_t[:], xt32[q])

                q_ps = conv_pp.tile([C, 2, W], F32, tag="q2")
                k_ps = conv_pp.tile([C, 2, W], F32, tag="k2")
                for t in range(9):
                    ky, kx = t // 3, t % 3
                    r = 2 * p + ky
                    nc.tensor.matmul(q_ps[:], wt_s[:, t, :],
                                     t1[:, r:r + 2, kx:kx + W],
                                     start=(t == 0), stop=(t == 8))
                    nc.tensor.matmul(k_ps[:], wt_s[:, t, :],
                                     t2[:, r:r + 2, kx:kx + W],
                                     start=(t == 0), stop=(t == 8))
                qk = qk_pool.tile([C, 2, 2, W], BF16)   # [c, row, q/k, w]
                nc.scalar.activation(qk[:, :, 0, :], q_ps[:], AF.Identity,
                                     bias=bias_s[:], scale=1.0)
                nc.scalar.activation(qk[:, :, 1, :], k_ps[:], AF.Identity,
                                     bias=bias_s[:], scale=1.0)
                state[q] = (qk, xtb_t, xt32_t)

            def emit_attn(q):
                """width attention + finalize + store for pair q."""
                qk, xtb_t, xt32_t = state.pop(q)
                lo_t = out_pool.tile([WC, 2, 2, C], F32, tag="lo")
                ro_t = out_pool.tile([WC, 2, 2, C], F32, tag="ro")
                for rr in range(2):
                    # --- S = Q^T K and St = K^T Q, chunked over W ---
                    s_ps = s_pp.tile([WC, 2, W], F32)
                    st_ps = st_pp.tile([WC, 2, W], F32)
                    for wc in range(2):
                        nc.tensor.matmul(s_ps[:, wc, :],
                                         qk[:, rr, 0, bass.ts(wc, WC)],
                                         qk[:, rr, 1, :],
                                         start=True, stop=True)
                        nc.tensor.matmul(st_ps[:, wc, :],
                                         qk[:, rr, 1, bass.ts(wc, WC)],
                                         qk[:, rr, 0, :],
                                         start=True, stop=True)

                    # --- E = exp(S), Et = exp(St) ---
                    e_t = e_pool.tile([WC, 4, W], BF16)
                    nc.scalar.activation(e_t[:, 0:2, :], s_ps[:], AF.Exp)
                    nc.scalar.activation(e_t[:, 2:4, :], st_ps[:], AF.Exp)

                    # --- apply (unnormalized) + ones-column row-sums ---
                    m_ps = m_pp.tile([WC, 4, WC + 1], F32)
                    for wc in range(2):
                        for vc in range(2):
                            nc.tensor.matmul(
                                m_ps[:, wc, :],
                                e_t[:, 2 + vc, bass.ts(wc, WC)],
                                xtb_t[:, rr, 2 + vc, :],
                                start=(vc == 0), stop=(vc == 1))
                    for vc in range(2):
                        for wc in range(2):
                            nc.tensor.matmul(
                                m_ps[:, 2 + vc, :],
                                e_t[:, wc, bass.ts(vc, WC)],
                                xtb_t[:, rr, wc, :],
                                start=(wc == 0), stop=(wc == 1))

                    rcp_t = rcp_pool.tile([WC, 4], F32)
                    nc.vector.reciprocal(rcp_t[:], m_ps[:, :, WC:WC + 1])

                    # --- finalize: out = base^T + M * (1/rs) ---
                    for wc in range(2):
                        nc.vector.scalar_tensor_tensor(
                            lo_t[:, rr, wc, :], m_ps[:, wc, 0:C],
                            rcp_t[:, wc:wc + 1], xt32_t[:, rr, wc, :],
                            op0=ALU.mult, op1=ALU.add)
                        nc.vector.scalar_tensor_tensor(
                            ro_t[:, rr, wc, :], m_ps[:, 2 + wc, 0:C],
                            rcp_t[:, 2 + wc:3 + wc], xt32_t[:, rr, 2 + wc, :],
                            op0=ALU.mult, op1=ALU.add)
                out_eng = (nc.gpsimd if os.environ.get("KV2_GPSIMD_OUT")
                           else nc.sync)
                out_eng.dma_start(loT[q], lo_t[:])
                out_eng.dma_start(roT[q], ro_t[:])

            # software pipeline: conv runs one pair ahead of attention
            emit_conv(0)
            for q in range(NPAIR):
                if q + 1 < NPAIR:
                    emit_conv(q + 1)
                emit_attn(q)

    nc.compile()
    return nc


def _prepare_inputs(low1, low2, conv_w, conv_b):
    low1 = np.asarray(low1, dtype=np.float32)
    low2 = np.asarray(low2, dtype=np.float32)
    conv_w = np.asarray(conv_w, dtype=np.float32)
    conv_b = np.asarray(conv_b, dtype=np.float32)

    xp1 = np.zeros((B, C, H + 2, W + 2), np.float32)
    xp1[:, :, 1:-1, 1:-1] = low1
    xp2 = np.zeros((B, C, H + 2, W + 2), np.float32)
    xp2[:, :, 1:-1, 1:-1] = low2

    wt = np.ascontiguousarray(conv_w.transpose(1, 2, 3, 0).reshape(C, 9, C))
    bias = np.ascontiguousarray(conv_b.reshape(C, 1))

    in_maps = []
    for k in range(NCORES):
        b, half = k // 2, k % 2
        r0 = half * HL

        def make_chunks(xp):
            out = np.zeros((NCHUNK, C, CROWS, WP), np.float32)
            for j in range(NCHUNK):
                lo = r0 + 14 * j
                hi = min(lo + CROWS, H + 2)
                out[j, :, :hi - lo, :] = xp[b, :, lo:hi, :]
            return out

        x1ck = make_chunks(xp1)
        x2ck = make_chunks(xp2)

        # transposed [h, w', slot, c] for both tensors; slot 0,1=low1T, 2,3=low2T
        l1t = low1[b, :, r0:r0 + HL, :].transpose(1, 2, 0)   # [h, w, c]
        l2t = low2[b, :, r0:r0 + HL, :].transpose(1, 2, 0)
        a1 = l1t.reshape(HL, 2, WC, C).transpose(0, 2, 1, 3)  # [h, w', wc, c]
        a2 = l2t.reshape(HL, 2, WC, C).transpose(0, 2, 1, 3)
        xt = np.concatenate([a1, a2], axis=2)                 # [h, w', 4, c]
        # pair-batch: [pair, w', row, slot, c]
        xt32 = np.ascontiguousarray(
            xt.reshape(NPAIR, 2, WC, 4, C).transpose(0, 2, 1, 3, 4))
        xtb = np.concatenate(
            [xt32, np.ones((NPAIR, WC, 2, 4, 1), np.float32)],
            axis=4).astype(ml_dtypes.bfloat16)

        in_maps.append({
            "x1c": x1ck,
            "x2c": x2ck,
            "xtb": np.ascontiguousarray(xtb),
            "xt32": xt32,
            "wt": wt,
            "bias": bias,
        })
    return in_maps


def _assemble(results):
    left = np.empty((B, C, H, W), np.float32)
    right = np.empty((B, C, H, W), np.float32)
    for k in range(NCORES):
        b, half = k // 2, k % 2
        r0 = half * HL
        for name, dst in (("loT", left), ("roT", right)):
            arr = results[k][name]                  # [pair, w', row, wc, c]
            # -> [c, pair, row, wc, w'] -> [c, h, w]
            dst[b, :, r0:r0 + HL, :] = (
                arr.transpose(4, 0, 2, 3, 1).reshape(C, HL, W))
    return left, right


def _run(inputs, trace=False):
    if trace:
        _install_profile_hook()
    if "nc" not in _CACHE:
        _CACHE["nc"] = _build()
    nc = _CACHE["nc"]
    in_maps = _prepare_inputs(**inputs)
    res = bass_utils.run_bass_kernel_spmd(
        nc, in_maps, core_ids=list(range(NCORES)), trace=trace)
    left, right = _assemble(res.results)
    return (left, right), res


def kernel(**inputs):
    out, _ = _run(inputs, trace=False)
    return out


# revision 12
# speedup vs baseline: 1.5095x; 1.1552x over previous
"""Trainium2 Bass kernel for width-axis cross attention (sparse_attention problem).

reference semantics:
  Q = conv3x3(low1, w, b); K = conv3x3(low2, w, b)
  score[b,h,w,v] = sum_c Q[b,c,h,w] * K[b,c,h,v]
  A_left  = softmax(score, axis=-1)            (relu is identity on softmax)
  A_right = softmax(score^T, axis=-1)
  left  = low1 + einsum('bhwv,bchv->bchw', A_left,  low2)
  right = low2 + einsum('bhwv,bchv->bchw', A_right, low1)

Sharding: data-parallel over (batch, H-half) -> 8 shards, no cross-core comm.

Per-core dataflow (96 rows, processed in row pairs):
 - conv as 9 accumulating f32r matmuls per tensor, 2 output rows per matmul
   (shared weight loads), PSUM -> SBUF bf16 with bias via ScalarE.
 - S = Q^T K and St = K^T Q in bf16; exp via ScalarE (one [96,384] op per side).
 - apply matmuls in bf16 against host-pre-transposed inputs, with an extra
   ones-column producing the softmax row-sums for free.
 - finalize = (M * 1/rs) + base^T in one fused VectorE op; outputs stored
   width-transposed, host un-transposes.
"""

import os
import sys

for _p in ("/opt/trn_rl_repo", "/root/.axon_site/_ro/trn_rl_repo"):
    if os.path.isdir(_p) and _p not in sys.path:
        sys.path.append(_p)

import numpy as np
import ml_dtypes

import concourse.bacc as bacc
import concourse.bass as bass
import concourse.tile as tile
from concourse import mybir
from concourse import bass_utils

B, C, H, W = 4, 96, 192, 192
NCORES = 8
HL = H // 2          # local rows per core
WP = W + 2           # width-padded
WC = W // 2          # 96-wide chunk of the W axis
NPAIR = HL // 2      # 48 row pairs
PAIRS_PER_CHUNK = 7
NCHUNK = -(-NPAIR // PAIRS_PER_CHUNK)        # 7
CROWS = 2 * PAIRS_PER_CHUNK + 2              # 16 rows per input chunk (1 halo each side)

F32 = mybir.dt.float32
F32R = mybir.dt.float32r
BF16 = mybir.dt.bfloat16
AF = mybir.ActivationFunctionType
ALU = mybir.AluOpType

CONV_BF16 = bool(os.environ.get("KV2_CONV_BF16"))
CONV_DT = BF16 if CONV_BF16 else F32R
CONV_NP = None  # set in _prepare_inputs

_CACHE = {}


def _install_profile_hook():
    """Register the axon NTFF profiling hook (missing from this image's antenv)."""
    if _CACHE.get("hook_done"):
        return
    _CACHE["hook_done"] = True
    import types
    import antenv

    if "antenv.axon_hooks" not in sys.modules:
        mod = types.ModuleType("antenv.axon_hooks")
        _h = {"fn": None}
        mod.set_axon_ntff_profile_hook = lambda fn: _h.__setitem__("fn", fn)
        mod.get_axon_ntff_profile_hook = lambda: _h["fn"]
        sys.modules["antenv.axon_hooks"] = mod
        antenv.axon_hooks = mod
    mod = sys.modules["antenv.axon_hooks"]
    try:
        from trn_agent_boot.trn_boot import _ntff_profile_via_ctypes

        hook = _ntff_profile_via_ctypes("/opt/axon/libaxon_pjrt.so")
        if hook is not None:
            mod.set_axon_ntff_profile_hook(hook)
    except Exception as e:  # profiling is best-effort
        print(f"profile hook install failed: {e}", file=sys.stderr)
    # avoid remote artifact uploads from the profiling path
    bass_utils.upload_artifacts = lambda tmpdir: "local://" + str(tmpdir)


def _build():
    """Build + compile the per-core Bass module (identical on all 8 cores)."""
    nc = bacc.Bacc("TRN2", target_bir_lowering=False, debug=False,
                   num_devices=NCORES)

    # inputs (per core)
    x1c = nc.dram_tensor("x1c", [NCHUNK, C, CROWS, WP], CONV_DT,
                         kind="ExternalInput").ap()
    x2c = nc.dram_tensor("x2c", [NCHUNK, C, CROWS, WP], CONV_DT,
                         kind="ExternalInput").ap()
    # [pair, w', row, slot, col]; slots 0,1 = low1T w-chunks, 2,3 = low2T
    # xtb has a 97th all-ones column (bf16) for free softmax row-sums.
    xtb = nc.dram_tensor("xtb", [NPAIR, WC, 2, 4, WC + 1], BF16,
                         kind="ExternalInput").ap()
    xt32 = nc.dram_tensor("xt32", [NPAIR, WC, 2, 4, WC], F32,
                          kind="ExternalInput").ap()
    wt = nc.dram_tensor("wt", [C, 9, C], CONV_DT, kind="ExternalInput").ap()
    bias = nc.dram_tensor("bias", [C, 1], F32, kind="ExternalInput").ap()
    # outputs, transposed layout: [pair, w', row, wc, c]
    loT = nc.dram_tensor("loT", [NPAIR, WC, 2, 2, C], F32,
                         kind="ExternalOutput").ap()
    roT = nc.dram_tensor("roT", [NPAIR, WC, 2, 2, C], F32,
                         kind="ExternalOutput").ap()

    with tile.TileContext(nc) as tc:
        with (
            tc.tile_pool(name="wpool", bufs=1) as wpool,
            tc.tile_pool(name="chunks", bufs=2) as chunk_pool,
            tc.tile_pool(name="xtbp", bufs=3) as xtb_pool,
            tc.tile_pool(name="xt32p", bufs=3) as xt32_pool,
            tc.tile_pool(name="qkp", bufs=2) as qk_pool,
            tc.tile_pool(name="ep", bufs=3) as e_pool,
            tc.tile_pool(name="rcpp", bufs=3) as rcp_pool,
            tc.tile_pool(name="outp", bufs=3) as out_pool,
            tc.tile_pool(name="convps", bufs=1, space="PSUM") as conv_pp,
            tc.tile_pool(name="sps", bufs=2, space="PSUM") as s_pp,
            tc.tile_pool(name="stps", bufs=2, space="PSUM") as st_pp,
            tc.tile_pool(name="mps", bufs=2, space="PSUM") as m_pp,
        ):
            wt_s = wpool.tile([C, 9, C], CONV_DT)
            nc.sync.dma_start(wt_s[:], wt)
            bias_s = wpool.tile([C, 1], F32)
            nc.sync.dma_start(bias_s[:], bias)

            ch_t = {}

            def load_chunk(j):
                t1 = chunk_pool.tile([C, CROWS, WP], CONV_DT, tag="x1")
                nc.sync.dma_start(t1[:], x1c[j])
                t2 = chunk_pool.tile([C, CROWS, WP], CONV_DT, tag="x2")
                nc.sync.dma_start(t2[:], x2c[j])
                ch_t[j] = (t1, t2)

            load_chunk(0)
            state = {}

            def emit_conv(q):
                """conv3x3 + bias/cast for pair q; prefetch DMAs for pair q."""
                j, p = divmod(q, PAIRS_PER_CHUNK)
                if p == 0 and j + 1 < NCHUNK:
                    load_chunk(j + 1)
                t1, t2 = ch_t[j]

                xtb_t = xtb_pool.tile([WC, 2, 4, WC + 1], BF16)
                nc.sync.dma_start(xtb_t[:], xtb[q])
                xt32_t = xt32_pool.tile([WC, 2, 4, WC], F32)
                nc.sync.dma_start(xt32_t[:], xt32[q])

                q_ps = conv_pp.tile([C, 2, W], F32, tag="q2")
                k_ps = conv_pp.tile([C, 2, W], F32, tag="k2")
                for t in range(9):
                    ky, kx = t // 3, t % 3
                    r = 2 * p + ky
                    nc.tensor.matmul(q_ps[:], wt_s[:, t, :],
                                     t1[:, r:r + 2, kx:kx + W],
                                     start=(t == 0), stop=(t == 8))
                    nc.tensor.matmul(k_ps[:], wt_s[:, t, :],
                                     t2[:, r:r + 2, kx:kx + W],
                                     start=(t == 0), stop=(t == 8))
                qk = qk_pool.tile([C, 2, 2, W], BF16)   # [c, row, q/k, w]
                nc.scalar.activation(qk[:, :, 0, :], q_ps[:], AF.Identity,
                                     bias=bias_s[:], scale=1.0)
                nc.scalar.activation(qk[:, :, 1, :], k_ps[:], AF.Identity,
                                     bias=bias_s[:], scale=1.0)
                state[q] = (qk, xtb_t, xt32_t)

            def emit_attn(q):
                """width attention + finalize + store for pair q."""
                qk, xtb_t, xt32_t = state.pop(q)
                lo_t = out_pool.tile([WC, 2, 2, C], F32, tag="lo")
                ro_t = out_pool.tile([WC, 2, 2, C], F32, tag="ro")
                for rr in range(2):
                    # --- S = Q^T K and St = K^T Q, chunked over W ---
                    s_ps = s_pp.tile([WC, 2, W], F32)
                    st_ps = st_pp.tile([WC, 2, W], F32)
                    for wc in range(2):
                        nc.tensor.matmul(s_ps[:, wc, :],
                                         qk[:, rr, 0, bass.ts(wc, WC)],
                                         qk[:, rr, 1, :],
                                         start=True, stop=True)
                        nc.tensor.matmul(st_ps[:, wc, :],
                                         qk[:, rr, 1, bass.ts(wc, WC)],
                                         qk[:, rr, 0, :],
                                         start=True, stop=True)

                    # --- E = exp(S), Et = exp(St) ---
                    e_t = e_pool.tile([WC, 4, W], BF16)
                    nc.scalar.activation(e_t[:, 0:2, :], s_ps[:], AF.Exp)
                    nc.scalar.activation(e_t[:, 2:4, :], st_ps[:], AF.Exp)

                    # --- apply (unnormalized) + ones-column row-sums ---
                    m_ps = m_pp.tile([WC, 4, WC + 1], F32)
                    for wc in range(2):
                        for vc in range(2):
                            nc.tensor.matmul(
                                m_ps[:, wc, :],
                                e_t[:, 2 + vc, bass.ts(wc, WC)],
                                xtb_t[:, rr, 2 + vc, :],
                                start=(vc == 0), stop=(vc == 1))
                    for vc in range(2):
                        for wc in range(2):
                            nc.tensor.matmul(
                                m_ps[:, 2 + vc, :],
                                e_t[:, wc, bass.ts(vc, WC)],
                                xtb_t[:, rr, wc, :],
                                start=(wc == 0), stop=(wc == 1))

                    rcp_t = rcp_pool.tile([WC, 4], F32)
                    nc.vector.reciprocal(rcp_t[:], m_ps[:, :, WC:WC + 1])

                    # --- finalize: out = base^T + M * (1/rs) ---
                    for wc in range(2):
                        nc.vector.scalar_tensor_tensor(
                            lo_t[:, rr, wc, :], m_ps[:, wc, 0:C],
                            rcp_t[:, wc:wc + 1], xt32_t[:, rr, wc, :],
                            op0=ALU.mult, op1=ALU.add)
                        nc.vector.scalar_tensor_tensor(
                            ro_t[:, rr, wc, :], m_ps[:, 2 + wc, 0:C],
                            rcp_t[:, 2 + wc:3 + wc], xt32_t[:, rr, 2 + wc, :],
                            op0=ALU.mult, op1=ALU.add)
                out_eng = (nc.gpsimd if os.environ.get("KV2_GPSIMD_OUT")
                           else nc.sync)
                out_eng.dma_start(loT[q], lo_t[:])
                out_eng.dma_start(roT[q], ro_t[:])

            # software pipeline: conv runs one pair ahead of attention
            emit_conv(0)
            for q in range(NPAIR):
                if q + 1 < NPAIR:
                    emit_conv(q + 1)
                emit_attn(q)

    nc.compile()
    return nc


def _prepare_inputs(low1, low2, conv_w, conv_b):
    low1 = np.asarray(low1, dtype=np.float32)
    low2 = np.asarray(low2, dtype=np.float32)
    conv_w = np.asarray(conv_w, dtype=np.float32)
    conv_b = np.asarray(conv_b, dtype=np.float32)

    xp1 = np.zeros((B, C, H + 2, W + 2), np.float32)
    xp1[:, :, 1:-1, 1:-1] = low1
    xp2 = np.zeros((B, C, H + 2, W + 2), np.float32)
    xp2[:, :, 1:-1, 1:-1] = low2

    wt = np.ascontiguousarray(conv_w.transpose(1, 2, 3, 0).reshape(C, 9, C))
    bias = np.ascontiguousarray(conv_b.reshape(C, 1))

    in_maps = []
    for k in range(NCORES):
        b, half = k // 2, k % 2
        r0 = half * HL

        def make_chunks(xp):
            out = np.zeros((NCHUNK, C, CROWS, WP), np.float32)
            for j in range(NCHUNK):
                lo = r0 + 14 * j
                hi = min(lo + CROWS, H + 2)
                out[j, :, :hi - lo, :] = xp[b, :, lo:hi, :]
            if CONV_BF16:
                return out.astype(ml_dtypes.bfloat16)
            return out

        x1ck = make_chunks(xp1)
        x2ck = make_chunks(xp2)

        # transposed [h, w', slot, c] for both tensors; slot 0,1=low1T, 2,3=low2T
        l1t = low1[b, :, r0:r0 + HL, :].transpose(1, 2, 0)   # [h, w, c]
        l2t = low2[b, :, r0:r0 + HL, :].transpose(1, 2, 0)
        a1 = l1t.reshape(HL, 2, WC, C).transpose(0, 2, 1, 3)  # [h, w', wc, c]
        a2 = l2t.reshape(HL, 2, WC, C).transpose(0, 2, 1, 3)
        xt = np.concatenate([a1, a2], axis=2)                 # [h, w', 4, c]
        # pair-batch: [pair, w', row, slot, c]
        xt32 = np.ascontiguousarray(
            xt.reshape(NPAIR, 2, WC, 4, C).transpose(0, 2, 1, 3, 4))
        xtb = np.concatenate(
            [xt32, np.ones((NPAIR, WC, 2, 4, 1), np.float32)],
            axis=4).astype(ml_dtypes.bfloat16)

        in_maps.append({
            "x1c": x1ck,
            "x2c": x2ck,
            "xtb": np.ascontiguousarray(xtb),
            "xt32": xt32,
            "wt": wt.astype(ml_dtypes.bfloat16) if CONV_BF16 else wt,
            "bias": bias,
        })
    return in_maps


def _assemble(results):
    left = np.empty((B, C, H, W), np.float32)
    right = np.empty((B, C, H, W), np.float32)
    for k in range(NCORES):
        b, half = k // 2, k % 2
        r0 = half * HL
        for name, dst in (("loT", left), ("roT", right)):
            arr = results[k][name]                  # [pair, w', row, wc, c]
            # -> [c, pair, row, wc, w'] -> [c, h, w]
            dst[b, :, r0:r0 + HL, :] = (
                arr.transpose(4, 0, 2, 3, 1).reshape(C, HL, W))
    return left, right


def _run(inputs, trace=False):
    if trace:
        _install_profile_hook()
    if "nc" not in _CACHE:
        _CACHE["nc"] = _build()
    nc = _CACHE["nc"]
    in_maps = _prepare_inputs(**inputs)
    res = bass_utils.run_bass_kernel_spmd(
        nc, in_maps, core_ids=list(range(NCORES)), trace=trace)
    left, right = _assemble(res.results)
    return (left, right), res


def kernel(**inputs):
    out, _ = _run(inputs, trace=False)
    return out


# revision 14
# speedup vs baseline: 1.5190x; 1.0063x over previous
"""Trainium2 Bass kernel for width-axis cross attention (sparse_attention problem).

reference semantics:
  Q = conv3x3(low1, w, b); K = conv3x3(low2, w, b)
  score[b,h,w,v] = sum_c Q[b,c,h,w] * K[b,c,h,v]
  A_left  = softmax(score, axis=-1)            (relu is identity on softmax)
  A_right = softmax(score^T, axis=-1)
  left  = low1 + einsum('bhwv,bchv->bchw', A_left,  low2)
  right = low2 + einsum('bhwv,bchv->bchw', A_right, low1)

Sharding: data-parallel over (batch, H-half) -> 8 shards, no cross-core comm.

Per-core dataflow (96 rows, processed in row pairs):
 - conv as 9 accumulating f32r matmuls per tensor, 2 output rows per matmul
   (shared weight loads), PSUM -> SBUF bf16 with bias via ScalarE.
 - S = Q^T K and St = K^T Q in bf16; exp via ScalarE (one [96,384] op per side).
 - apply matmuls in bf16 against host-pre-transposed inputs, with an extra
   ones-column producing the softmax row-sums for free.
 - finalize = (M * 1/rs) + base^T in one fused VectorE op; outputs stored
   width-transposed, host un-transposes.
"""

import os
import sys

for _p in ("/opt/trn_rl_repo", "/root/.axon_site/_ro/trn_rl_repo"):
    if os.path.isdir(_p) and _p not in sys.path:
        sys.path.append(_p)

import numpy as np
import ml_dtypes

import concourse.bacc as bacc
import concourse.bass as bass
import concourse.tile as tile
from concourse import mybir
from concourse import bass_utils

B, C, H, W = 4, 96, 192, 192
NCORES = 8
HL = H // 2          # local rows per core
WP = W + 2           # width-padded
WC = W // 2          # 96-wide chunk of the W axis
NPAIR = HL // 2      # 48 row pairs
PAIRS_PER_CHUNK = 7
NCHUNK = -(-NPAIR // PAIRS_PER_CHUNK)        # 7
CROWS = 2 * PAIRS_PER_CHUNK + 2              # 16 rows per input chunk (1 halo each side)

F32 = mybir.dt.float32
F32R = mybir.dt.float32r
BF16 = mybir.dt.bfloat16
FP16 = mybir.dt.float16
AF = mybir.ActivationFunctionType
ALU = mybir.AluOpType

# precision mode: fp16 (default; 10 mantissa bits), bf16, or f32r conv
PREC = os.environ.get("KV2_PREC", "fp16")
if PREC == "fp16":
    LP_DT, LP_NP, CONV_DT, CONV_NP = FP16, np.float16, FP16, np.float16
elif PREC == "bf16":
    LP_DT, LP_NP, CONV_DT, CONV_NP = (BF16, ml_dtypes.bfloat16,
                                      BF16, ml_dtypes.bfloat16)
else:  # f32r conv, bf16 attention
    LP_DT, LP_NP, CONV_DT, CONV_NP = BF16, ml_dtypes.bfloat16, F32R, np.float32
ESHIFT = 12.0 if PREC == "fp16" else 0.0

_CACHE = {}


def _install_profile_hook():
    """Register the axon NTFF profiling hook (missing from this image's antenv)."""
    if _CACHE.get("hook_done"):
        return
    _CACHE["hook_done"] = True
    import types
    import antenv

    if "antenv.axon_hooks" not in sys.modules:
        mod = types.ModuleType("antenv.axon_hooks")
        _h = {"fn": None}
        mod.set_axon_ntff_profile_hook = lambda fn: _h.__setitem__("fn", fn)
        mod.get_axon_ntff_profile_hook = lambda: _h["fn"]
        sys.modules["antenv.axon_hooks"] = mod
        antenv.axon_hooks = mod
    mod = sys.modules["antenv.axon_hooks"]
    try:
        from trn_agent_boot.trn_boot import _ntff_profile_via_ctypes

        hook = _ntff_profile_via_ctypes("/opt/axon/libaxon_pjrt.so")
        if hook is not None:
            mod.set_axon_ntff_profile_hook(hook)
    except Exception as e:  # profiling is best-effort
        print(f"profile hook install failed: {e}", file=sys.stderr)
    # avoid remote artifact uploads from the profiling path
    bass_utils.upload_artifacts = lambda tmpdir: "local://" + str(tmpdir)


def _build():
    """Build + compile the per-core Bass module (identical on all 8 cores)."""
    nc = bacc.Bacc("TRN2", target_bir_lowering=False, debug=False,
                   num_devices=NCORES)

    # inputs (per core)
    x1c = nc.dram_tensor("x1c", [NCHUNK, C, CROWS, WP], CONV_DT,
                         kind="ExternalInput").ap()
    x2c = nc.dram_tensor("x2c", [NCHUNK, C, CROWS, WP], CONV_DT,
                         kind="ExternalInput").ap()
    # [pair, w', row, slot, col]; slots 0,1 = low1T w-chunks, 2,3 = low2T
    # xtb has a 97th all-ones column (bf16) for free softmax row-sums.
    xtb = nc.dram_tensor("xtb", [NPAIR, WC, 2, 4, WC + 1], LP_DT,
                         kind="ExternalInput").ap()
    xt32 = nc.dram_tensor("xt32", [NPAIR, WC, 2, 4, WC], F32,
                          kind="ExternalInput").ap()
    wt = nc.dram_tensor("wt", [C, 9, C], CONV_DT, kind="ExternalInput").ap()
    bias = nc.dram_tensor("bias", [C, 1], F32, kind="ExternalInput").ap()
    # outputs, transposed layout: [pair, w', row, wc, c]
    loT = nc.dram_tensor("loT", [NPAIR, WC, 2, 2, C], F32,
                         kind="ExternalOutput").ap()
    roT = nc.dram_tensor("roT", [NPAIR, WC, 2, 2, C], F32,
                         kind="ExternalOutput").ap()

    with tile.TileContext(nc) as tc:
        with (
            tc.tile_pool(name="wpool", bufs=1) as wpool,
            tc.tile_pool(name="chunks", bufs=2) as chunk_pool,
            tc.tile_pool(name="xtbp", bufs=3) as xtb_pool,
            tc.tile_pool(name="xt32p", bufs=3) as xt32_pool,
            tc.tile_pool(name="qkp", bufs=2) as qk_pool,
            tc.tile_pool(name="ep", bufs=3) as e_pool,
            tc.tile_pool(name="rcpp", bufs=3) as rcp_pool,
            tc.tile_pool(name="outp", bufs=3) as out_pool,
            tc.tile_pool(name="convps", bufs=1, space="PSUM") as conv_pp,
            tc.tile_pool(name="sps", bufs=2, space="PSUM") as s_pp,
            tc.tile_pool(name="stps", bufs=2, space="PSUM") as st_pp,
            tc.tile_pool(name="mps", bufs=2, space="PSUM") as m_pp,
        ):
            wt_s = wpool.tile([C, 9, C], CONV_DT)
            nc.sync.dma_start(wt_s[:], wt)
            bias_s = wpool.tile([C, 1], F32)
            nc.sync.dma_start(bias_s[:], bias)
            eshift_s = wpool.tile([WC, 1], F32)
            nc.gpsimd.memset(eshift_s[:], -ESHIFT)

            ch_t = {}

            def load_chunk(j):
                t1 = chunk_pool.tile([C, CROWS, WP], CONV_DT, tag="x1")
                nc.sync.dma_start(t1[:], x1c[j])
                t2 = chunk_pool.tile([C, CROWS, WP], CONV_DT, tag="x2")
                nc.sync.dma_start(t2[:], x2c[j])
                ch_t[j] = (t1, t2)

            load_chunk(0)
            state = {}

            def emit_conv(q):
                """conv3x3 + bias/cast for pair q; prefetch DMAs for pair q."""
                j, p = divmod(q, PAIRS_PER_CHUNK)
                if p == 0 and j + 1 < NCHUNK:
                    load_chunk(j + 1)
                t1, t2 = ch_t[j]

                xtb_t = xtb_pool.tile([WC, 2, 4, WC + 1], LP_DT)
                nc.sync.dma_start(xtb_t[:], xtb[q])
                xt32_t = xt32_pool.tile([WC, 2, 4, WC], F32)
                nc.sync.dma_start(xt32_t[:], xt32[q])

                q_ps = conv_pp.tile([C, 2, W], F32, tag="q2")
                k_ps = conv_pp.tile([C, 2, W], F32, tag="k2")
                for t in range(9):
                    ky, kx = t // 3, t % 3
                    r = 2 * p + ky
                    nc.tensor.matmul(q_ps[:], wt_s[:, t, :],
                                     t1[:, r:r + 2, kx:kx + W],
                                     start=(t == 0), stop=(t == 8))
                    nc.tensor.matmul(k_ps[:], wt_s[:, t, :],
                                     t2[:, r:r + 2, kx:kx + W],
                                     start=(t == 0), stop=(t == 8))
                qk = qk_pool.tile([C, 2, 2, W], LP_DT)   # [c, row, q/k, w]
                nc.scalar.activation(qk[:, :, 0, :], q_ps[:], AF.Identity,
                                     bias=bias_s[:], scale=1.0)
                nc.scalar.activation(qk[:, :, 1, :], k_ps[:], AF.Identity,
                                     bias=bias_s[:], scale=1.0)
                state[q] = (qk, xtb_t, xt32_t)

            def emit_attn(q):
                """width attention + finalize + store for pair q."""
                qk, xtb_t, xt32_t = state.pop(q)
                lo_t = out_pool.tile([WC, 2, 2, C], F32, tag="lo")
                ro_t = out_pool.tile([WC, 2, 2, C], F32, tag="ro")
                for rr in range(2):
                    # --- S = Q^T K and St = K^T Q, chunked over W ---
                    s_ps = s_pp.tile([WC, 2, W], F32)
                    st_ps = st_pp.tile([WC, 2, W], F32)
                    for wc in range(2):
                        nc.tensor.matmul(s_ps[:, wc, :],
                                         qk[:, rr, 0, bass.ts(wc, WC)],
                                         qk[:, rr, 1, :],
                                         start=True, stop=True)
                        nc.tensor.matmul(st_ps[:, wc, :],
                                         qk[:, rr, 1, bass.ts(wc, WC)],
                                         qk[:, rr, 0, :],
                                         start=True, stop=True)

                    # --- E = exp(S), Et = exp(St) ---
                    e_t = e_pool.tile([WC, 4, W], LP_DT)
                    nc.scalar.activation(e_t[:, 0:2, :], s_ps[:], AF.Exp, bias=eshift_s[:])
                    nc.scalar.activation(e_t[:, 2:4, :], st_ps[:], AF.Exp, bias=eshift_s[:])

                    # --- apply (unnormalized) + ones-column row-sums ---
                    m_ps = m_pp.tile([WC, 4, WC + 1], F32)
                    for wc in range(2):
                        for vc in range(2):
                            nc.tensor.matmul(
                                m_ps[:, wc, :],
                                e_t[:, 2 + vc, bass.ts(wc, WC)],
                                xtb_t[:, rr, 2 + vc, :],
                                start=(vc == 0), stop=(vc == 1))
                    for vc in range(2):
                        for wc in range(2):
                            nc.tensor.matmul(
                                m_ps[:, 2 + vc, :],
                                e_t[:, wc, bass.ts(vc, WC)],
                                xtb_t[:, rr, wc, :],
                                start=(wc == 0), stop=(wc == 1))

                    rcp_t = rcp_pool.tile([WC, 4], F32)
                    nc.vector.reciprocal(rcp_t[:], m_ps[:, :, WC:WC + 1])

                    # --- finalize: out = base^T + M * (1/rs) ---
                    for wc in range(2):
                        nc.vector.scalar_tensor_tensor(
                            lo_t[:, rr, wc, :], m_ps[:, wc, 0:C],
                            rcp_t[:, wc:wc + 1], xt32_t[:, rr, wc, :],
                            op0=ALU.mult, op1=ALU.add)
                        nc.vector.scalar_tensor_tensor(
                            ro_t[:, rr, wc, :], m_ps[:, 2 + wc, 0:C],
                            rcp_t[:, 2 + wc:3 + wc], xt32_t[:, rr, 2 + wc, :],
                            op0=ALU.mult, op1=ALU.add)
                out_eng = (nc.gpsimd if os.environ.get("KV2_GPSIMD_OUT")
                           else nc.sync)
                out_eng.dma_start(loT[q], lo_t[:])
                out_eng.dma_start(roT[q], ro_t[:])

            # software pipeline: conv runs one pair ahead of attention
            emit_conv(0)
            for q in range(NPAIR):
                if q + 1 < NPAIR:
                    emit_conv(q + 1)
                emit_attn(q)

    nc.compile()
    return nc


def _prepare_inputs(low1, low2, conv_w, conv_b):
    low1 = np.asarray(low1, dtype=np.float32)
    low2 = np.asarray(low2, dtype=np.float32)
    conv_w = np.asarray(conv_w, dtype=np.float32)
    conv_b = np.asarray(conv_b, dtype=np.float32)

    xp1 = np.zeros((B, C, H + 2, W + 2), np.float32)
    xp1[:, :, 1:-1, 1:-1] = low1
    xp2 = np.zeros((B, C, H + 2, W + 2), np.float32)
    xp2[:, :, 1:-1, 1:-1] = low2

    wt = np.ascontiguousarray(conv_w.transpose(1, 2, 3, 0).reshape(C, 9, C))
    bias = np.ascontiguousarray(conv_b.reshape(C, 1))

    in_maps = []
    for k in range(NCORES):
        b, half = k // 2, k % 2
        r0 = half * HL

        def make_chunks(xp):
            out = np.zeros((NCHUNK, C, CROWS, WP), np.float32)
            for j in range(NCHUNK):
                lo = r0 + 14 * j
                hi = min(lo + CROWS, H + 2)
                out[j, :, :hi - lo, :] = xp[b, :, lo:hi, :]
            if CONV_NP is not np.float32:
                return out.astype(CONV_NP)
            return out

        x1ck = make_chunks(xp1)
        x2ck = make_chunks(xp2)

        # transposed [h, w', slot, c] for both tensors; slot 0,1=low1T, 2,3=low2T
        l1t = low1[b, :, r0:r0 + HL, :].transpose(1, 2, 0)   # [h, w, c]
        l2t = low2[b, :, r0:r0 + HL, :].transpose(1, 2, 0)
        a1 = l1t.reshape(HL, 2, WC, C).transpose(0, 2, 1, 3)  # [h, w', wc, c]
        a2 = l2t.reshape(HL, 2, WC, C).transpose(0, 2, 1, 3)
        xt = np.concatenate([a1, a2], axis=2)                 # [h, w', 4, c]
        # pair-batch: [pair, w', row, slot, c]
        xt32 = np.ascontiguousarray(
            xt.reshape(NPAIR, 2, WC, 4, C).transpose(0, 2, 1, 3, 4))
        xtb = np.concatenate(
            [xt32, np.ones((NPAIR, WC, 2, 4, 1), np.float32)],
            axis=4).astype(LP_NP)

        in_maps.append({
            "x1c": x1ck,
            "x2c": x2ck,
            "xtb": np.ascontiguousarray(xtb),
            "xt32": xt32,
            "wt": wt.astype(CONV_NP) if CONV_NP is not np.float32 else wt,
            "bias": bias,
        })
    return in_maps


def _assemble(results):
    left = np.empty((B, C, H, W), np.float32)
    right = np.empty((B, C, H, W), np.float32)
    for k in range(NCORES):
        b, half = k // 2, k % 2
        r0 = half * HL
        for name, dst in (("loT", left), ("roT", right)):
            arr = results[k][name]                  # [pair, w', row, wc, c]
            # -> [c, pair, row, wc, w'] -> [c, h, w]
            dst[b, :, r0:r0 + HL, :] = (
                arr.transpose(4, 0, 2, 3, 1).reshape(C, HL, W))
    return left, right


def _run(inputs, trace=False):
    if trace:
        _install_profile_hook()
    if "nc" not in _CACHE:
        _CACHE["nc"] = _build()
    nc = _CACHE["nc"]
    in_maps = _prepare_inputs(**inputs)
    res = bass_utils.run_bass_kernel_spmd(
        nc, in_maps, core_ids=list(range(NCORES)), trace=trace)
    left, right = _assemble(res.results)
    return (left, right), res


def kernel(**inputs):
    out, _ = _run(inputs, trace=False)
    return out


# revision 15
# speedup vs baseline: 1.5320x; 1.0085x over previous
"""Trainium2 Bass kernel for width-axis cross attention (sparse_attention problem).

reference semantics:
  Q = conv3x3(low1, w, b); K = conv3x3(low2, w, b)
  score[b,h,w,v] = sum_c Q[b,c,h,w] * K[b,c,h,v]
  A_left  = softmax(score, axis=-1)            (relu is identity on softmax)
  A_right = softmax(score^T, axis=-1)
  left  = low1 + einsum('bhwv,bchv->bchw', A_left,  low2)
  right = low2 + einsum('bhwv,bchv->bchw', A_right, low1)

Sharding: data-parallel over (batch, H-half) -> 8 shards, no cross-core comm.

Per-core dataflow (96 rows, processed in row pairs):
 - conv as 9 accumulating f32r matmuls per tensor, 2 output rows per matmul
   (shared weight loads), PSUM -> SBUF bf16 with bias via ScalarE.
 - S = Q^T K and St = K^T Q in bf16; exp via ScalarE (one [96,384] op per side).
 - apply matmuls in bf16 against host-pre-transposed inputs, with an extra
   ones-column producing the softmax row-sums for free.
 - finalize = (M * 1/rs) + base^T in one fused VectorE op; outputs stored
   width-transposed, host un-transposes.
"""

import os
import sys

for _p in ("/opt/trn_rl_repo", "/root/.axon_site/_ro/trn_rl_repo"):
    if os.path.isdir(_p) and _p not in sys.path:
        sys.path.append(_p)

import numpy as np
import ml_dtypes

import concourse.bacc as bacc
import concourse.bass as bass
import concourse.tile as tile
from concourse import mybir
from concourse import bass_utils

B, C, H, W = 4, 96, 192, 192
NCORES = 8
HL = H // 2          # local rows per core
WP = W + 2           # width-padded
WC = W // 2          # 96-wide chunk of the W axis
NPAIR = HL // 2      # 48 row pairs
PAIRS_PER_CHUNK = 4
NCHUNK = -(-NPAIR // PAIRS_PER_CHUNK)        # 12
CROWS = 2 * PAIRS_PER_CHUNK + 2              # 16 rows per input chunk (1 halo each side)

F32 = mybir.dt.float32
F32R = mybir.dt.float32r
BF16 = mybir.dt.bfloat16
FP16 = mybir.dt.float16
AF = mybir.ActivationFunctionType
ALU = mybir.AluOpType

# precision mode: fp16 (default; 10 mantissa bits), bf16, or f32r conv
PREC = os.environ.get("KV2_PREC", "fp16")
if PREC == "fp16":
    LP_DT, LP_NP, CONV_DT, CONV_NP = FP16, np.float16, FP16, np.float16
elif PREC == "bf16":
    LP_DT, LP_NP, CONV_DT, CONV_NP = (BF16, ml_dtypes.bfloat16,
                                      BF16, ml_dtypes.bfloat16)
else:  # f32r conv, bf16 attention
    LP_DT, LP_NP, CONV_DT, CONV_NP = BF16, ml_dtypes.bfloat16, F32R, np.float32
ESHIFT = 12.0 if PREC == "fp16" else 0.0

_CACHE = {}


def _install_profile_hook():
    """Register the axon NTFF profiling hook (missing from this image's antenv)."""
    if _CACHE.get("hook_done"):
        return
    _CACHE["hook_done"] = True
    import types
    import antenv

    if "antenv.axon_hooks" not in sys.modules:
        mod = types.ModuleType("antenv.axon_hooks")
        _h = {"fn": None}
        mod.set_axon_ntff_profile_hook = lambda fn: _h.__setitem__("fn", fn)
        mod.get_axon_ntff_profile_hook = lambda: _h["fn"]
        sys.modules["antenv.axon_hooks"] = mod
        antenv.axon_hooks = mod
    mod = sys.modules["antenv.axon_hooks"]
    try:
        from trn_agent_boot.trn_boot import _ntff_profile_via_ctypes

        hook = _ntff_profile_via_ctypes("/opt/axon/libaxon_pjrt.so")
        if hook is not None:
            mod.set_axon_ntff_profile_hook(hook)
    except Exception as e:  # profiling is best-effort
        print(f"profile hook install failed: {e}", file=sys.stderr)
    # avoid remote artifact uploads from the profiling path
    bass_utils.upload_artifacts = lambda tmpdir: "local://" + str(tmpdir)


def _build():
    """Build + compile the per-core Bass module (identical on all 8 cores)."""
    nc = bacc.Bacc("TRN2", target_bir_lowering=False, debug=False,
                   num_devices=NCORES)

    # inputs (per core)
    x1c = nc.dram_tensor("x1c", [NCHUNK, C, CROWS, WP], CONV_DT,
                         kind="ExternalInput").ap()
    x2c = nc.dram_tensor("x2c", [NCHUNK, C, CROWS, WP], CONV_DT,
                         kind="ExternalInput").ap()
    # [pair, w', row, slot, col]; slots 0,1 = low1T w-chunks, 2,3 = low2T
    # xtb has a 97th all-ones column (bf16) for free softmax row-sums.
    xtb = nc.dram_tensor("xtb", [NPAIR, WC, 2, 4, WC + 1], LP_DT,
                         kind="ExternalInput").ap()
    xt32 = nc.dram_tensor("xt32", [NPAIR, WC, 2, 4, WC], F32,
                          kind="ExternalInput").ap()
    wt = nc.dram_tensor("wt", [C, 9, C], CONV_DT, kind="ExternalInput").ap()
    bias = nc.dram_tensor("bias", [C, 1], F32, kind="ExternalInput").ap()
    # outputs, transposed layout: [pair, w', row, wc, c]
    loT = nc.dram_tensor("loT", [NPAIR, WC, 2, 2, C], F32,
                         kind="ExternalOutput").ap()
    roT = nc.dram_tensor("roT", [NPAIR, WC, 2, 2, C], F32,
                         kind="ExternalOutput").ap()

    with tile.TileContext(nc) as tc:
        with (
            tc.tile_pool(name="wpool", bufs=1) as wpool,
            tc.tile_pool(name="chunks", bufs=2) as chunk_pool,
            tc.tile_pool(name="xtbp", bufs=3) as xtb_pool,
            tc.tile_pool(name="xt32p", bufs=3) as xt32_pool,
            tc.tile_pool(name="qkp", bufs=2) as qk_pool,
            tc.tile_pool(name="ep", bufs=3) as e_pool,
            tc.tile_pool(name="rcpp", bufs=3) as rcp_pool,
            tc.tile_pool(name="outp", bufs=3) as out_pool,
            tc.tile_pool(name="convps", bufs=1, space="PSUM") as conv_pp,
            tc.tile_pool(name="sps", bufs=2, space="PSUM") as s_pp,
            tc.tile_pool(name="stps", bufs=2, space="PSUM") as st_pp,
            tc.tile_pool(name="mps", bufs=2, space="PSUM") as m_pp,
        ):
            wt_s = wpool.tile([C, 9, C], CONV_DT)
            nc.sync.dma_start(wt_s[:], wt)
            bias_s = wpool.tile([C, 1], F32)
            nc.sync.dma_start(bias_s[:], bias)
            eshift_s = wpool.tile([WC, 1], F32)
            nc.gpsimd.memset(eshift_s[:], -ESHIFT)

            ch_t = {}

            def load_chunk(j):
                t1 = chunk_pool.tile([C, CROWS, WP], CONV_DT, tag="x1")
                nc.sync.dma_start(t1[:], x1c[j])
                t2 = chunk_pool.tile([C, CROWS, WP], CONV_DT, tag="x2")
                nc.sync.dma_start(t2[:], x2c[j])
                ch_t[j] = (t1, t2)

            load_chunk(0)
            state = {}

            def emit_conv(q):
                """conv3x3 + bias/cast for pair q; prefetch DMAs for pair q."""
                j, p = divmod(q, PAIRS_PER_CHUNK)
                if p == 0 and j + 1 < NCHUNK:
                    load_chunk(j + 1)
                t1, t2 = ch_t[j]

                xtb_t = xtb_pool.tile([WC, 2, 4, WC + 1], LP_DT)
                nc.sync.dma_start(xtb_t[:], xtb[q])
                xt32_t = xt32_pool.tile([WC, 2, 4, WC], F32)
                nc.sync.dma_start(xt32_t[:], xt32[q])

                q_ps = conv_pp.tile([C, 2, W], F32, tag="q2")
                k_ps = conv_pp.tile([C, 2, W], F32, tag="k2")
                for t in range(9):
                    ky, kx = t // 3, t % 3
                    r = 2 * p + ky
                    nc.tensor.matmul(q_ps[:], wt_s[:, t, :],
                                     t1[:, r:r + 2, kx:kx + W],
                                     start=(t == 0), stop=(t == 8))
                    nc.tensor.matmul(k_ps[:], wt_s[:, t, :],
                                     t2[:, r:r + 2, kx:kx + W],
                                     start=(t == 0), stop=(t == 8))
                qk = qk_pool.tile([C, 2, 2, W], LP_DT)   # [c, row, q/k, w]
                nc.scalar.activation(qk[:, :, 0, :], q_ps[:], AF.Identity,
                                     bias=bias_s[:], scale=1.0)
                nc.scalar.activation(qk[:, :, 1, :], k_ps[:], AF.Identity,
                                     bias=bias_s[:], scale=1.0)
                state[q] = (qk, xtb_t, xt32_t)

            def emit_attn(q):
                """width attention + finalize + store for pair q."""
                qk, xtb_t, xt32_t = state.pop(q)
                lo_t = out_pool.tile([WC, 2, 2, C], F32, tag="lo")
                ro_t = out_pool.tile([WC, 2, 2, C], F32, tag="ro")
                for rr in range(2):
                    # --- S = Q^T K and St = K^T Q, chunked over W ---
                    s_ps = s_pp.tile([WC, 2, W], F32)
                    st_ps = st_pp.tile([WC, 2, W], F32)
                    for wc in range(2):
                        nc.tensor.matmul(s_ps[:, wc, :],
                                         qk[:, rr, 0, bass.ts(wc, WC)],
                                         qk[:, rr, 1, :],
                                         start=True, stop=True)
                        nc.tensor.matmul(st_ps[:, wc, :],
                                         qk[:, rr, 1, bass.ts(wc, WC)],
                                         qk[:, rr, 0, :],
                                         start=True, stop=True)

                    # --- E = exp(S), Et = exp(St) ---
                    e_t = e_pool.tile([WC, 4, W], LP_DT)
                    nc.scalar.activation(e_t[:, 0:2, :], s_ps[:], AF.Exp, bias=eshift_s[:])
                    nc.scalar.activation(e_t[:, 2:4, :], st_ps[:], AF.Exp, bias=eshift_s[:])

                    # --- apply (unnormalized) + ones-column row-sums ---
                    m_ps = m_pp.tile([WC, 4, WC + 1], F32)
                    for wc in range(2):
                        for vc in range(2):
                            nc.tensor.matmul(
                                m_ps[:, wc, :],
                                e_t[:, 2 + vc, bass.ts(wc, WC)],
                                xtb_t[:, rr, 2 + vc, :],
                                start=(vc == 0), stop=(vc == 1))
                    for vc in range(2):
                        for wc in range(2):
                            nc.tensor.matmul(
                                m_ps[:, 2 + vc, :],
                                e_t[:, wc, bass.ts(vc, WC)],
                                xtb_t[:, rr, wc, :],
                                start=(wc == 0), stop=(wc == 1))

                    rcp_t = rcp_pool.tile([WC, 4], F32)
                    nc.vector.reciprocal(rcp_t[:], m_ps[:, :, WC:WC + 1])

                    # --- finalize: out = base^T + M * (1/rs) ---
                    for wc in range(2):
                        nc.vector.scalar_tensor_tensor(
                            lo_t[:, rr, wc, :], m_ps[:, wc, 0:C],
                            rcp_t[:, wc:wc + 1], xt32_t[:, rr, wc, :],
                            op0=ALU.mult, op1=ALU.add)
                        nc.vector.scalar_tensor_tensor(
                            ro_t[:, rr, wc, :], m_ps[:, 2 + wc, 0:C],
                            rcp_t[:, 2 + wc:3 + wc], xt32_t[:, rr, 2 + wc, :],
                            op0=ALU.mult, op1=ALU.add)
                out_eng = (nc.gpsimd if os.environ.get("KV2_GPSIMD_OUT")
                           else nc.sync)
                out_eng.dma_start(loT[q], lo_t[:])
                out_eng.dma_start(roT[q], ro_t[:])

            # software pipeline: conv runs one pair ahead of attention
            emit_conv(0)
            for q in range(NPAIR):
                if q + 1 < NPAIR:
                    emit_conv(q + 1)
                emit_attn(q)

    nc.compile()
    return nc


def _prepare_inputs(low1, low2, conv_w, conv_b):
    low1 = np.asarray(low1, dtype=np.float32)
    low2 = np.asarray(low2, dtype=np.float32)
    conv_w = np.asarray(conv_w, dtype=np.float32)
    conv_b = np.asarray(conv_b, dtype=np.float32)

    xp1 = np.zeros((B, C, H + 2, W + 2), np.float32)
    xp1[:, :, 1:-1, 1:-1] = low1
    xp2 = np.zeros((B, C, H + 2, W + 2), np.float32)
    xp2[:, :, 1:-1, 1:-1] = low2

    wt = np.ascontiguousarray(conv_w.transpose(1, 2, 3, 0).reshape(C, 9, C))
    bias = np.ascontiguousarray(conv_b.reshape(C, 1))

    in_maps = []
    for k in range(NCORES):
        b, half = k // 2, k % 2
        r0 = half * HL

        def make_chunks(xp):
            out = np.zeros((NCHUNK, C, CROWS, WP), np.float32)
            for j in range(NCHUNK):
                lo = r0 + 2 * PAIRS_PER_CHUNK * j
                hi = min(lo + CROWS, H + 2)
                out[j, :, :hi - lo, :] = xp[b, :, lo:hi, :]
            if CONV_NP is not np.float32:
                return out.astype(CONV_NP)
            return out

        x1ck = make_chunks(xp1)
        x2ck = make_chunks(xp2)

        # transposed [h, w', slot, c] for both tensors; slot 0,1=low1T, 2,3=low2T
        l1t = low1[b, :, r0:r0 + HL, :].transpose(1, 2, 0)   # [h, w, c]
        l2t = low2[b, :, r0:r0 + HL, :].transpose(1, 2, 0)
        a1 = l1t.reshape(HL, 2, WC, C).transpose(0, 2, 1, 3)  # [h, w', wc, c]
        a2 = l2t.reshape(HL, 2, WC, C).transpose(0, 2, 1, 3)
        xt = np.concatenate([a1, a2], axis=2)                 # [h, w', 4, c]
        # pair-batch: [pair, w', row, slot, c]
        xt32 = np.ascontiguousarray(
            xt.reshape(NPAIR, 2, WC, 4, C).transpose(0, 2, 1, 3, 4))
        xtb = np.concatenate(
            [xt32, np.ones((NPAIR, WC, 2, 4, 1), np.float32)],
            axis=4).astype(LP_NP)

        in_maps.append({
            "x1c": x1ck,
            "x2c": x2ck,
            "xtb": np.ascontiguousarray(xtb),
            "xt32": xt32,
            "wt": wt.astype(CONV_NP) if CONV_NP is not np.float32 else wt,
            "bias": bias,
        })
    return in_maps


def _assemble(results):
    left = np.empty((B, C, H, W), np.float32)
    right = np.empty((B, C, H, W), np.float32)
    for k in range(NCORES):
        b, half = k // 2, k % 2
        r0 = half * HL
        for name, dst in (("loT", left), ("roT", right)):
            arr = results[k][name]                  # [pair, w', row, wc, c]
            # -> [c, pair, row, wc, w'] -> [c, h, w]
            dst[b, :, r0:r0 + HL, :] = (
                arr.transpose(4, 0, 2, 3, 1).reshape(C, HL, W))
    return left, right


def _run(inputs, trace=False):
    if trace:
        _install_profile_hook()
    if "nc" not in _CACHE:
        _CACHE["nc"] = _build()
    nc = _CACHE["nc"]
    in_maps = _prepare_inputs(**inputs)
    res = bass_utils.run_bass_kernel_spmd(
        nc, in_maps, core_ids=list(range(NCORES)), trace=trace)
    left, right = _assemble(res.results)
    return (left, right), res


def kernel(**inputs):
    out, _ = _run(inputs, trace=False)
    return out
